# revision 6
# baseline (speedup 1.0000x reference)
"""DeeperGCN (28-layer GENConv, softmax aggregation) on 8 Trainium2 NeuronCores.

Strategy
--------
Nodes are partitioned contiguously across the 8 cores (1250 each); each core
owns all edges whose *destination* lands in its range.  The per-edge message
exp(t*(relu(z_src)+eps)) and its numerator weight are pure functions of the
source node, so each layer reduces to:

  1. node-local:  z = relu(LN(h)); ez = exp(t*(z+eps)); q = ez*(z+eps)
  2. AllGather the bf16 [ez|q] node table (N x 128) into HBM
  3. dma_gather per-edge 256B rows from that table (sorted by dst group)
  4. segment-sum via PE matmuls with one-hot selector matrices S
     (psum[dst,0:64]=den, psum[dst,64:128]=num), agg = num/(den+eps)
  5. f32 MLP (Linear->LN->ReLU->Linear) via PE transposes, residual into h

The softmax max-subtraction is dropped: logits are bounded (z is LN'd +
relu'd), so exp never overflows and the result is identical within f32
rounding.  Aggregation payload is bf16 (measured end-to-end rel err ~5e-4);
everything else is f32.
"""

import os
import sys

import numpy as np
import ml_dtypes

sys.path.insert(0, "/opt/trn_rl_repo")

import concourse.bacc as bacc
import concourse.tile as tile
import concourse.mybir as mybir
from concourse import library_config
from concourse.bass_utils import run_bass_kernel_spmd

bf16 = ml_dtypes.bfloat16
F32 = mybir.dt.float32
BF16 = mybir.dt.bfloat16
I16 = mybir.dt.int16
ALU = mybir.AluOpType
AF = mybir.ActivationFunctionType

N = 10000
E = 160000
IN = 128
H = 64
H2 = 128
OUT = 112
L = int(os.environ.get("GNN_LAYERS", "28"))
NCORES = 8
NPC = N // NCORES          # 1250 nodes per core
G = 10                     # 128-node groups per core (last has 98)
LN_EPS = 1e-5
MSG_EPS = 1e-7
SM_EPS = 1e-16


def _preprocess(edge_index):
    """Partition edges by destination core/group; build per-core gather index
    arrays and one-hot selector matrices."""
    src = np.ascontiguousarray(edge_index[0]).astype(np.int64)
    dst = np.ascontiguousarray(edge_index[1]).astype(np.int64)
    core = dst // NPC
    local = dst - core * NPC
    grp = local // 128                      # 0..9

    cnt = np.zeros((NCORES, G), np.int64)
    np.add.at(cnt, (core, grp), 1)
    cpg = [int(np.ceil(cnt[:, g].max() / 128)) for g in range(G)]
    off = np.concatenate([[0], np.cumsum(cpg)])
    tc = int(off[-1])                       # total chunks per core
    ni = tc * 128                           # total gather slots per core

    idx_tiles = []
    s_tiles = []
    for k in range(NCORES):
        idx_flat = np.zeros(ni, np.int16)
        s_mat = np.zeros((128, tc, 128), np.float32)
        for g in range(G):
            m = (core == k) & (grp == g)
            e_src = src[m]
            e_col = (local[m] - g * 128).astype(np.int64)
            p = np.arange(e_src.size)
            c = off[g] + p // 128
            lane = p % 128
            idx_flat[c * 128 + lane] = e_src.astype(np.int16)
            s_mat[lane, c, e_col] = 1.0
        # wrap: gather position i -> [i % 16, i // 16]; replicate 8x across
        # the 128 partitions (one 16-partition stripe per GPSIMD Q7 core)
        iw = idx_flat.reshape(ni // 16, 16).T.copy()
        idx_tiles.append(np.tile(iw, (8, 1)))
        s_tiles.append(s_mat.reshape(128, tc * 128).astype(bf16))
    return cpg, off, tc, idx_tiles, s_tiles


def _build(params, cpg, off, tc):
    """Trace + compile the SPMD Bass program.  `params` holds the actual
    weight values: trivial (ones/zeros) affine params are folded away at
    trace time."""
    nc = bacc.Bacc("TRN2", target_bir_lowering=False, debug=False,
                   num_devices=NCORES)

    ln_g, ln_b = params["ln_g"], params["ln_b"]
    mlp_g, mlp_b = params["mlp_g"], params["mlp_b"]
    t_vals = params["t"]
    b1v, b2v = params["b1"], params["b2"]
    enc_bv, head_bv = params["enc_b"], params["head_b"]

    ln_triv = [bool(np.all(ln_g[l] == 1) and np.all(ln_b[l] == 0)) for l in range(L)]
    mlp_triv = [bool(np.all(mlp_g[l] == 1) and np.all(mlp_b[l] == 0)) for l in range(L)]
    b1_triv = [bool(np.all(b1v[l] == 0)) for l in range(L)]
    b2_triv = [bool(np.all(b2v[l] == 0)) for l in range(L)]
    encb_triv = bool(np.all(enc_bv == 0))
    headb_triv = bool(np.all(head_bv == 0))
    need_brows = (not all(b1_triv)) or (not all(b2_triv)) or (not encb_triv) or (not headb_triv)
    need_lnrep = not all(ln_triv)
    need_mlprep = not all(mlp_triv)

    # ---- I/O -------------------------------------------------------------
    p_xT = nc.dram_tensor("xT", [128, G * 128], F32, kind="ExternalInput")
    p_s = nc.dram_tensor("s", [128, tc * 128], BF16, kind="ExternalInput")
    p_idx = nc.dram_tensor("idx", [128, tc * 8], I16, kind="ExternalInput")
    p_w1 = nc.dram_tensor("w1", [H, L * H2], F32, kind="ExternalInput")
    p_w2 = nc.dram_tensor("w2", [H2, L * H], F32, kind="ExternalInput")
    p_encw = nc.dram_tensor("encw", [IN, H], F32, kind="ExternalInput")
    p_headw = nc.dram_tensor("headw", [H, OUT], F32, kind="ExternalInput")
    p_id = nc.dram_tensor("id128", [128, 128], F32, kind="ExternalInput")
    p_out = nc.dram_tensor("out", [NPC, OUT], F32, kind="ExternalOutput")
    if need_brows:
        p_brows = nc.dram_tensor("brows", [1, L * H2 + L * H + H + OUT], F32,
                                 kind="ExternalInput")
    if need_lnrep:
        p_lnrep = nc.dram_tensor("lnrep", [128, L * 2 * H], F32, kind="ExternalInput")
    if need_mlprep:
        p_mlprep = nc.dram_tensor("mlprep", [128, L * 2 * H2], F32, kind="ExternalInput")

    cc_in = nc.dram_tensor("cc_in", [NPC, H2], BF16)
    cc_out = nc.dram_tensor("cc_out", [N, H2], BF16, addr_space="Shared")

    rows_of = [min(128, NPC - 128 * g) for g in range(G)]   # 128,...,128,98

    with tile.TileContext(nc) as tcx:
        with tcx.tile_pool(name="big", bufs=1) as big, \
             tcx.tile_pool(name="node", bufs=1) as node, \
             tcx.tile_pool(name="stat", bufs=2) as stat, \
             tcx.tile_pool(name="pseg", bufs=2, space="PSUM") as pseg, \
             tcx.tile_pool(name="ps1", bufs=1, space="PSUM") as ps1p, \
             tcx.tile_pool(name="pst", bufs=3, space="PSUM") as pst:

            nc.gpsimd.load_library(library_config.mlp)

            t_lneps = node.tile([128, 1], F32)
            nc.vector.memset(t_lneps[:], LN_EPS)

            # ---- resident loads -----------------------------------------
            t_s = big.tile([128, tc, 128], BF16)
            nc.sync.dma_start(t_s[:], p_s.ap().rearrange("p (c f) -> p c f", c=tc))
            t_idx = big.tile([128, tc * 8], I16)
            nc.sync.dma_start(t_idx[:], p_idx[:])
            t_w1 = big.tile([H, L * H2], F32)
            nc.sync.dma_start(t_w1[:], p_w1[:])
            t_w2 = big.tile([H2, L * H], F32)
            nc.sync.dma_start(t_w2[:], p_w2[:])
            t_encw = big.tile([IN, H], F32)
            nc.sync.dma_start(t_encw[:], p_encw[:])
            t_headw = big.tile([H, OUT], F32)
            nc.sync.dma_start(t_headw[:], p_headw[:])
            t_id = big.tile([128, 128], F32)
            nc.sync.dma_start(t_id[:], p_id[:])
            t_xT = big.tile([128, G * 128], F32)
            nc.sync.dma_start(t_xT[:], p_xT[:])
            if need_brows:
                t_brows = big.tile([1, L * H2 + L * H + H + OUT], F32)
                nc.sync.dma_start(t_brows[:], p_brows[:])
                t_ones = big.tile([1, 128], F32)
                nc.vector.memset(t_ones[:], 1.0)
            if need_lnrep:
                t_lnrep = big.tile([128, L * 2 * H], F32)
                nc.sync.dma_start(t_lnrep[:], p_lnrep[:])
            if need_mlprep:
                t_mlprep = big.tile([128, L * 2 * H2], F32)
                nc.sync.dma_start(t_mlprep[:], p_mlprep[:])

            # ---- persistent node state ----------------------------------
            t_h = node.tile([128, G, H], F32)
            t_z = node.tile([128, G, H], F32)
            t_zp = node.tile([128, G, H], F32)
            t_ez = node.tile([128, G, H], F32)
            t_ezq = node.tile([128, G, H2], BF16)
            t_edges = big.tile([128, tc, 128], BF16)
            t_dn = node.tile([128, G, H2], F32)
            t_dne = node.tile([128, G, H], F32)
            t_rec = node.tile([128, G, H], F32)
            t_tmp = node.tile([128, G, H], F32)
            t_outsb = node.tile([128, G, H], F32)
            t_outT = node.tile([H, G, 128], F32)
            t_sq2 = node.tile([128, G, H2], F32)
            t_nrm2 = node.tile([128, G, H2], F32)
            t_u = node.tile([128, G, H2], F32)
            t_uT = node.tile([H2, G, 128], F32)

            def rank1_bias(psum_ap, row_ap, n):
                # psum += ones^T @ row  (adds a per-channel row to every node)
                nc.tensor.matmul(psum_ap, lhsT=t_ones[:, 0:128], rhs=row_ap,
                                 start=False, stop=True, skip_group_check=True)

            # ---- encoder: h = x @ enc_W (+enc_b) ------------------------
            for g in range(G):
                ps_e = pst.tile([128, H], F32, tag="pst")
                nc.tensor.matmul(ps_e[:], lhsT=t_xT[:, 128 * g:128 * (g + 1)],
                                 rhs=t_encw[:], start=True, stop=encb_triv)
                if not encb_triv:
                    rank1_bias(ps_e[:], t_brows[:, L * H2 + L * H:L * H2 + L * H + H], H)
                nc.scalar.copy(t_h[:, g, :], ps_e[:])

            # ---- per-layer stats tiles ----------------------------------
            for l in range(L):
                w1_l = t_w1[:, l * H2:(l + 1) * H2]
                w2_l = t_w2[:, l * H:(l + 1) * H]
                t_l = float(t_vals[l])

                # ===== phase A: node-local LN + exp ======================
                st1 = stat.tile([128, G], F32, tag="st1")
                sqh = stat.tile([128, G, H], F32, tag="sqh")
                st2 = stat.tile([128, G], F32, tag="st2")
                mu = stat.tile([128, G], F32, tag="mu")
                ex2 = stat.tile([128, G], F32, tag="ex2")
                var = stat.tile([128, G], F32, tag="var")
                sd = stat.tile([128, G], F32, tag="sd")
                rstd = stat.tile([128, G], F32, tag="rstd")
                nrm = stat.tile([128, G, H], F32, tag="nrm")

                nc.vector.reduce_sum(st1[:], t_h[:], axis=mybir.AxisListType.X)
                nc.scalar.square(sqh[:], t_h[:])
                nc.vector.reduce_sum(st2[:], sqh[:], axis=mybir.AxisListType.X)
                nc.vector.tensor_scalar_mul(mu[:], st1[:], 1.0 / H)
                nc.vector.tensor_scalar_mul(ex2[:], st2[:], 1.0 / H)
                nc.vector.tensor_mul(var[:], mu[:], mu[:])
                nc.vector.tensor_sub(var[:], ex2[:], var[:])
                nc.scalar.activation(sd[:], var[:], AF.Sqrt, bias=t_lneps[:])
                nc.vector.reciprocal_approx_fast(out=rstd[:], in_=sd[:])
                for g in range(G):
                    nc.vector.tensor_scalar(nrm[:, g, :], t_h[:, g, :],
                                            mu[:, g:g + 1], rstd[:, g:g + 1],
                                            ALU.subtract, ALU.mult)
                if not ln_triv[l]:
                    nc.vector.tensor_mul(
                        nrm[:], nrm[:],
                        t_lnrep[:, l * 2 * H:l * 2 * H + H]
                        .rearrange("p h -> p 1 h").to_broadcast([128, G, H]))
                    nc.vector.tensor_add(
                        nrm[:], nrm[:],
                        t_lnrep[:, l * 2 * H + H:(l + 1) * 2 * H]
                        .rearrange("p h -> p 1 h").to_broadcast([128, G, H]))
                nc.scalar.activation(t_z[:], nrm[:], AF.Relu)
                nc.vector.tensor_scalar_add(t_zp[:], t_z[:], MSG_EPS)
                nc.scalar.activation(t_ez[:], t_zp[:], AF.Exp, scale=t_l)
                nc.vector.tensor_copy(t_ezq[:, :, 0:H], t_ez[:])
                nc.vector.tensor_mul(t_ezq[:, :, H:H2], t_ez[:], t_zp[:])

                # ===== phase B: share [ez|q] =============================
                for g in range(G):
                    r = rows_of[g]
                    nc.sync.dma_start(cc_in[128 * g:128 * g + r, :],
                                      t_ezq[0:r, g, :])
                nc.gpsimd.collective_compute(
                    "AllGather", ALU.bypass,
                    replica_groups=[list(range(NCORES))],
                    ins=[cc_in[:]], outs=[cc_out[:]])

                # ===== phase C: gather + segment matmuls =================
                for g in range(G):
                    npg = cpg[g]
                    o = off[g]
                    nc.gpsimd.dma_gather(
                        out_ap=t_edges[:, o:o + npg, :],
                        in_ap=cc_out[:],
                        idxs_ap=t_idx[:, o * 8:(o + npg) * 8],
                        num_idxs=npg * 128,
                        num_idxs_reg=npg * 128,
                        elem_size=H2,
                        single_packet=os.environ.get("GNN_SP", "0") == "1")
                    ps_g = pseg.tile([128, H2], F32, tag="pseg")
                    for j in range(npg):
                        nc.tensor.matmul(ps_g[:], lhsT=t_s[:, o + j, :],
                                         rhs=t_edges[:, o + j, :],
                                         start=(j == 0), stop=(j == npg - 1))
                    nc.scalar.copy(t_dn[:, g, :], ps_g[:])

                # ===== phase D: agg + MLP ================================
                nc.vector.tensor_scalar_add(t_dne[:], t_dn[:, :, 0:H], SM_EPS)
                nc.vector.reciprocal_approx_fast(out=t_rec[:], in_=t_dne[:])
                nc.vector.tensor_mul(t_tmp[:], t_dn[:, :, H:H2], t_rec[:])
                nc.vector.tensor_add(t_outsb[:], t_tmp[:], t_z[:])

                t_ps1 = ps1p.tile([128, G, H2], F32, tag="ps1")
                for g in range(G):
                    psT = pst.tile([H, 128], F32, tag="pst")
                    nc.tensor.transpose(psT[:], t_outsb[:, g, :], t_id[:])
                    nc.scalar.copy(t_outT[:, g, :], psT[:])
                    nc.tensor.matmul(t_ps1[:, g, :], lhsT=t_outT[:, g, :],
                                     rhs=w1_l, start=True, stop=b1_triv[l])
                    if not b1_triv[l]:
                        rank1_bias(t_ps1[:, g, :],
                                   t_brows[:, l * H2:(l + 1) * H2], H2)

                st1b = stat.tile([128, G], F32, tag="st1b")
                st2b = stat.tile([128, G], F32, tag="st2b")
                mu2 = stat.tile([128, G], F32, tag="mu2")
                ex2b = stat.tile([128, G], F32, tag="ex2b")
                var2 = stat.tile([128, G], F32, tag="var2")
                sd2 = stat.tile([128, G], F32, tag="sd2")
                rstd2 = stat.tile([128, G], F32, tag="rstd2")

                nc.vector.reduce_sum(st1b[:], t_ps1[:], axis=mybir.AxisListType.X)
                nc.scalar.square(t_sq2[:], t_ps1[:])
                nc.vector.reduce_sum(st2b[:], t_sq2[:], axis=mybir.AxisListType.X)
                nc.vector.tensor_scalar_mul(mu2[:], st1b[:], 1.0 / H2)
                nc.vector.tensor_scalar_mul(ex2b[:], st2b[:], 1.0 / H2)
                nc.vector.tensor_mul(var2[:], mu2[:], mu2[:])
                nc.vector.tensor_sub(var2[:], ex2b[:], var2[:])
                nc.scalar.activation(sd2[:], var2[:], AF.Sqrt, bias=t_lneps[:])
                nc.vector.reciprocal_approx_fast(out=rstd2[:], in_=sd2[:])
                for g in range(G):
                    nc.vector.tensor_scalar(t_nrm2[:, g, :], t_ps1[:, g, :],
                                            mu2[:, g:g + 1], rstd2[:, g:g + 1],
                                            ALU.subtract, ALU.mult)
                if not mlp_triv[l]:
                    nc.vector.tensor_mul(
                        t_nrm2[:], t_nrm2[:],
                        t_mlprep[:, l * 2 * H2:l * 2 * H2 + H2]
                        .rearrange("p h -> p 1 h").to_broadcast([128, G, H2]))
                    nc.vector.tensor_add(
                        t_nrm2[:], t_nrm2[:],
                        t_mlprep[:, l * 2 * H2 + H2:(l + 1) * 2 * H2]
                        .rearrange("p h -> p 1 h").to_broadcast([128, G, H2]))
                nc.scalar.activation(t_u[:], t_nrm2[:], AF.Relu)

                for g in range(G):
                    psT2 = pst.tile([128, 128], F32, tag="pst")
                    nc.tensor.transpose(psT2[:], t_u[:, g, :], t_id[:])
                    if g % 2 == 0:
                        nc.vector.tensor_copy(t_uT[:, g, :], psT2[:])
                    else:
                        nc.scalar.copy(t_uT[:, g, :], psT2[:])
                    ps2 = pst.tile([128, H], F32, tag="pst")
                    nc.tensor.matmul(ps2[:], lhsT=t_uT[:, g, :], rhs=w2_l,
                                     start=True, stop=b2_triv[l])
                    if not b2_triv[l]:
                        rank1_bias(ps2[:], t_brows[:, L * H2 + l * H:L * H2 + (l + 1) * H], H)
                    nc.vector.tensor_add(t_h[:, g, :], t_h[:, g, :], ps2[:])

            # ---- head: out = h @ head_W (+head_b) -----------------------
            for g in range(G):
                psT = pst.tile([H, 128], F32, tag="pst")
                nc.tensor.transpose(psT[:], t_h[:, g, :], t_id[:])
                nc.scalar.copy(t_outT[:, g, :], psT[:])
                ps_o = pst.tile([128, OUT], F32, tag="pst")
                nc.tensor.matmul(ps_o[:], lhsT=t_outT[:, g, :], rhs=t_headw[:],
                                 start=True, stop=headb_triv)
                if not headb_triv:
                    rank1_bias(ps_o[:], t_brows[:, L * H2 + L * H + H:], OUT)
                ou = stat.tile([128, OUT], F32, tag="ou")
                nc.vector.tensor_copy(ou[:], ps_o[:])
                r = rows_of[g]
                nc.sync.dma_start(p_out[128 * g:128 * g + r, :], ou[0:r, :])

    nc.compile()
    return nc


_CACHE = {}


def kernel(**inputs):
    x = np.asarray(inputs["x"], np.float32)
    edge_index = np.asarray(inputs["edge_index"]).astype(np.int64)
    params = {k: np.asarray(v, np.float32) for k, v in inputs.items()
              if k not in ("x", "edge_index")}

    cpg, off, tc, idx_tiles, s_tiles = _preprocess(edge_index)

    key = (tc, tuple(cpg),
           tuple(np.asarray(inputs["t"], np.float32).tolist()))
    nc = _CACHE.get(key)
    if nc is None:
        nc = _build(params, cpg, off, tc)
        _CACHE[key] = nc

    w1 = params["W1"][:L].transpose(1, 0, 2).reshape(H, L * H2)
    w2 = params["W2"][:L].transpose(1, 0, 2).reshape(H2, L * H)
    id128 = np.eye(128, dtype=np.float32)

    brows = np.concatenate([params["b1"][:L].reshape(-1), params["b2"][:L].reshape(-1),
                            params["enc_b"].reshape(-1),
                            params["head_b"].reshape(-1)])[None, :].astype(np.float32)
    lnrep = np.tile(np.concatenate([params["ln_g"][:L], params["ln_b"][:L]], axis=1)
                    .reshape(1, L * 2 * H), (128, 1)).astype(np.float32)
    mlprep = np.tile(np.concatenate([params["mlp_g"][:L], params["mlp_b"][:L]], axis=1)
                     .reshape(1, L * 2 * H2), (128, 1)).astype(np.float32)

    in_maps = []
    for k in range(NCORES):
        xs = x[k * NPC:(k + 1) * NPC]
        xT = np.zeros((128, G * 128), np.float32)
        xT[:, :NPC] = xs.T
        m = {
            "xT": xT,
            "s": s_tiles[k],
            "idx": idx_tiles[k],
            "w1": np.ascontiguousarray(w1),
            "w2": np.ascontiguousarray(w2),
            "encw": params["enc_W"],
            "headw": params["head_W"],
            "id128": id128,
        }
        names = {t.name for t in nc.m.functions[0].allocations
                 if hasattr(t, "name")}
        # optional params only exist when the build emitted them
        for nm, arr in (("brows", brows), ("lnrep", lnrep), ("mlprep", mlprep)):
            m[nm] = arr
        in_maps.append(m)

    # drop optional inputs the program doesn't declare
    declared = set()
    for alloc in nc.m.functions[0].allocations:
        if isinstance(alloc, mybir.MemoryLocationSet) and alloc.kind == "ExternalInput":
            declared.add(alloc.memorylocations[0].name)
    in_maps = [{k2: v for k2, v in m.items() if k2 in declared} for m in in_maps]

    if os.environ.get("GNN_SIM", "0") == "1":
        from concourse.bass_interp import MultiCoreSim
        sim = MultiCoreSim(nc, NCORES)
        for k in range(NCORES):
            for name, arr in in_maps[k].items():
                sim.cores[k].tensor(name)[:] = arr
        sim.simulate()
        out = np.concatenate([np.asarray(sim.cores[k].tensor("out"))
                              for k in range(NCORES)], axis=0)
        return out.astype(np.float32)

    trace = os.environ.get("GNN_TRACE", "0") == "1"
    res = run_bass_kernel_spmd(nc, in_maps, list(range(NCORES)), trace=trace)
    if trace and res.exec_time_ns is not None:
        print(f"HW exec time: {res.exec_time_ns} ns")

    out = np.concatenate([res.results[k]["out"] for k in range(NCORES)], axis=0)
    return out.astype(np.float32)


# revision 7
# speedup vs baseline: 1.5528x; 1.5528x over previous
"""DeeperGCN (28-layer GENConv, softmax aggregation) on 8 Trainium2 NeuronCores.

Strategy
--------
Nodes are partitioned contiguously across the 8 cores (1250 each); each core
owns all edges whose *destination* lands in its range.  The per-edge message
exp(t*(relu(z_src)+eps)) and its numerator weight are pure functions of the
source node, so each layer reduces to:

  1. node-local:  z = relu(LN(h)); ez = exp(t*(z+eps)); q = ez*(z+eps)
  2. AllGather the bf16 [ez|q] node table (N x 128) into HBM
  3. dma_gather per-edge 256B rows from that table (sorted by dst group)
  4. segment-sum via PE matmuls with one-hot selector matrices S
     (psum[dst,0:64]=den, psum[dst,64:128]=num), agg = num/(den+eps)
  5. f32 MLP (Linear->LN->ReLU->Linear) via PE transposes, residual into h

The softmax max-subtraction is dropped: logits are bounded (z is LN'd +
relu'd), so exp never overflows and the result is identical within f32
rounding.  Aggregation payload is bf16 (measured end-to-end rel err ~5e-4);
everything else is f32.
"""

import os
import sys

import numpy as np
import ml_dtypes

sys.path.insert(0, "/opt/trn_rl_repo")

import concourse.bacc as bacc
import concourse.tile as tile
import concourse.mybir as mybir
from concourse import library_config
from concourse.bass_utils import run_bass_kernel_spmd

bf16 = ml_dtypes.bfloat16
F32 = mybir.dt.float32
BF16 = mybir.dt.bfloat16
I16 = mybir.dt.int16
ALU = mybir.AluOpType
AF = mybir.ActivationFunctionType

N = 10000
E = 160000
IN = 128
H = 64
H2 = 128
OUT = 112
L = int(os.environ.get("GNN_LAYERS", "28"))
NCORES = 8
NPC = N // NCORES          # 1250 nodes per core
G = 10                     # 128-node groups per core (last has 98)
LN_EPS = 1e-5
MSG_EPS = 1e-7
SM_EPS = 1e-16


def _preprocess(edge_index):
    """Partition edges by destination core/group; build per-core gather index
    arrays and one-hot selector matrices."""
    src = np.ascontiguousarray(edge_index[0]).astype(np.int64)
    dst = np.ascontiguousarray(edge_index[1]).astype(np.int64)
    core = dst // NPC
    local = dst - core * NPC
    grp = local // 128                      # 0..9

    cnt = np.zeros((NCORES, G), np.int64)
    np.add.at(cnt, (core, grp), 1)
    cpg = [int(np.ceil(cnt[:, g].max() / 128)) for g in range(G)]
    off = np.concatenate([[0], np.cumsum(cpg)])
    tc = int(off[-1])                       # total chunks per core
    ni = tc * 128                           # total gather slots per core

    idx_tiles = []
    s_tiles = []
    for k in range(NCORES):
        idx_flat = np.zeros(ni, np.int16)
        s_mat = np.zeros((128, tc, 128), np.float32)
        for g in range(G):
            m = (core == k) & (grp == g)
            e_src = src[m]
            e_col = (local[m] - g * 128).astype(np.int64)
            p = np.arange(e_src.size)
            c = off[g] + p // 128
            lane = p % 128
            idx_flat[c * 128 + lane] = e_src.astype(np.int16)
            s_mat[lane, c, e_col] = 1.0
        # wrap: gather position i -> [i % 16, i // 16]; replicate 8x across
        # the 128 partitions (one 16-partition stripe per GPSIMD Q7 core)
        iw = idx_flat.reshape(ni // 16, 16).T.copy()
        idx_tiles.append(np.tile(iw, (8, 1)))
        s_tiles.append(s_mat.reshape(128, tc * 128).astype(bf16))
    return cpg, off, tc, idx_tiles, s_tiles


def _build(params, cpg, off, tc):
    """Trace + compile the SPMD Bass program.  `params` holds the actual
    weight values: trivial (ones/zeros) affine params are folded away at
    trace time."""
    nc = bacc.Bacc("TRN2", target_bir_lowering=False, debug=False,
                   num_devices=NCORES, num_swdge_queues=4)

    ln_g, ln_b = params["ln_g"], params["ln_b"]
    mlp_g, mlp_b = params["mlp_g"], params["mlp_b"]
    t_vals = params["t"]
    b1v, b2v = params["b1"], params["b2"]
    enc_bv, head_bv = params["enc_b"], params["head_b"]

    ln_triv = [bool(np.all(ln_g[l] == 1) and np.all(ln_b[l] == 0)) for l in range(L)]
    mlp_triv = [bool(np.all(mlp_g[l] == 1) and np.all(mlp_b[l] == 0)) for l in range(L)]
    b1_triv = [bool(np.all(b1v[l] == 0)) for l in range(L)]
    b2_triv = [bool(np.all(b2v[l] == 0)) for l in range(L)]
    encb_triv = bool(np.all(enc_bv == 0))
    headb_triv = bool(np.all(head_bv == 0))
    need_brows = (not all(b1_triv)) or (not all(b2_triv)) or (not encb_triv) or (not headb_triv)
    need_lnrep = not all(ln_triv)
    need_mlprep = not all(mlp_triv)

    # ---- I/O -------------------------------------------------------------
    p_xT = nc.dram_tensor("xT", [128, G * 128], F32, kind="ExternalInput")
    p_s = nc.dram_tensor("s", [128, tc * 128], BF16, kind="ExternalInput")
    p_idx = nc.dram_tensor("idx", [128, tc * 8], I16, kind="ExternalInput")
    p_w1 = nc.dram_tensor("w1", [H, L * H2], F32, kind="ExternalInput")
    p_w2 = nc.dram_tensor("w2", [H2, L * H], F32, kind="ExternalInput")
    p_encw = nc.dram_tensor("encw", [IN, H], F32, kind="ExternalInput")
    p_headw = nc.dram_tensor("headw", [H, OUT], F32, kind="ExternalInput")
    p_id = nc.dram_tensor("id128", [128, 128], F32, kind="ExternalInput")
    p_out = nc.dram_tensor("out", [NPC, OUT], F32, kind="ExternalOutput")
    if need_brows:
        p_brows = nc.dram_tensor("brows", [1, L * H2 + L * H + H + OUT], F32,
                                 kind="ExternalInput")
    if need_lnrep:
        p_lnrep = nc.dram_tensor("lnrep", [128, L * 2 * H], F32, kind="ExternalInput")
    if need_mlprep:
        p_mlprep = nc.dram_tensor("mlprep", [128, L * 2 * H2], F32, kind="ExternalInput")

    cc_in = nc.dram_tensor("cc_in", [NPC, H2], BF16)
    cc_out = nc.dram_tensor("cc_out", [N, H2], BF16, addr_space="Shared")

    rows_of = [min(128, NPC - 128 * g) for g in range(G)]   # 128,...,128,98

    with tile.TileContext(nc) as tcx:
        with tcx.tile_pool(name="big", bufs=1) as big, \
             tcx.tile_pool(name="node", bufs=1) as node, \
             tcx.tile_pool(name="stat", bufs=2) as stat, \
             tcx.tile_pool(name="pseg", bufs=2, space="PSUM") as pseg, \
             tcx.tile_pool(name="ps1", bufs=1, space="PSUM") as ps1p, \
             tcx.tile_pool(name="pst", bufs=3, space="PSUM") as pst:

            nc.gpsimd.load_library(library_config.mlp)

            t_lneps = node.tile([128, 1], F32)
            nc.vector.memset(t_lneps[:], LN_EPS)

            # ---- resident loads -----------------------------------------
            t_s = big.tile([128, tc, 128], BF16)
            nc.sync.dma_start(t_s[:], p_s.ap().rearrange("p (c f) -> p c f", c=tc))
            t_idx = big.tile([128, tc * 8], I16)
            nc.sync.dma_start(t_idx[:], p_idx[:])
            t_w1 = big.tile([H, L * H2], F32)
            nc.sync.dma_start(t_w1[:], p_w1[:])
            t_w2 = big.tile([H2, L * H], F32)
            nc.sync.dma_start(t_w2[:], p_w2[:])
            t_encw = big.tile([IN, H], F32)
            nc.sync.dma_start(t_encw[:], p_encw[:])
            t_headw = big.tile([H, OUT], F32)
            nc.sync.dma_start(t_headw[:], p_headw[:])
            t_id = big.tile([128, 128], F32)
            nc.sync.dma_start(t_id[:], p_id[:])
            t_xT = big.tile([128, G * 128], F32)
            nc.sync.dma_start(t_xT[:], p_xT[:])
            if need_brows:
                t_brows = big.tile([1, L * H2 + L * H + H + OUT], F32)
                nc.sync.dma_start(t_brows[:], p_brows[:])
                t_ones = big.tile([1, 128], F32)
                nc.vector.memset(t_ones[:], 1.0)
            if need_lnrep:
                t_lnrep = big.tile([128, L * 2 * H], F32)
                nc.sync.dma_start(t_lnrep[:], p_lnrep[:])
            if need_mlprep:
                t_mlprep = big.tile([128, L * 2 * H2], F32)
                nc.sync.dma_start(t_mlprep[:], p_mlprep[:])

            # ---- persistent node state ----------------------------------
            t_h = node.tile([128, G, H], F32)
            t_z = node.tile([128, G, H], F32)
            t_zp = node.tile([128, G, H], F32)
            t_ez = node.tile([128, G, H], F32)
            t_ezq = node.tile([128, G, H2], BF16)
            t_edges = big.tile([128, tc, 128], BF16)
            t_dn = node.tile([128, G, H2], F32)
            t_dne = node.tile([128, G, H], F32)
            t_rec = node.tile([128, G, H], F32)
            t_tmp = node.tile([128, G, H], F32)
            t_outsb = node.tile([128, G, H], F32)
            t_outT = node.tile([H, G, 128], F32)
            t_sq2 = node.tile([128, G, H2], F32)
            t_nrm2 = node.tile([128, G, H2], F32)
            t_u = node.tile([128, G, H2], F32)
            t_uT = node.tile([H2, G, 128], F32)

            def rank1_bias(psum_ap, row_ap, n):
                # psum += ones^T @ row  (adds a per-channel row to every node)
                nc.tensor.matmul(psum_ap, lhsT=t_ones[:, 0:128], rhs=row_ap,
                                 start=False, stop=True, skip_group_check=True)

            # ---- encoder: h = x @ enc_W (+enc_b) ------------------------
            for g in range(G):
                ps_e = pst.tile([128, H], F32, tag="pst")
                nc.tensor.matmul(ps_e[:], lhsT=t_xT[:, 128 * g:128 * (g + 1)],
                                 rhs=t_encw[:], start=True, stop=encb_triv)
                if not encb_triv:
                    rank1_bias(ps_e[:], t_brows[:, L * H2 + L * H:L * H2 + L * H + H], H)
                nc.scalar.copy(t_h[:, g, :], ps_e[:])

            # ---- per-layer stats tiles ----------------------------------
            for l in range(L):
                w1_l = t_w1[:, l * H2:(l + 1) * H2]
                w2_l = t_w2[:, l * H:(l + 1) * H]
                t_l = float(t_vals[l])

                # ===== phase A: node-local LN + exp ======================
                st1 = stat.tile([128, G], F32, tag="st1")
                sqh = stat.tile([128, G, H], F32, tag="sqh")
                st2 = stat.tile([128, G], F32, tag="st2")
                mu = stat.tile([128, G], F32, tag="mu")
                ex2 = stat.tile([128, G], F32, tag="ex2")
                var = stat.tile([128, G], F32, tag="var")
                sd = stat.tile([128, G], F32, tag="sd")
                rstd = stat.tile([128, G], F32, tag="rstd")
                nrm = stat.tile([128, G, H], F32, tag="nrm")

                nc.vector.reduce_sum(st1[:], t_h[:], axis=mybir.AxisListType.X)
                nc.scalar.square(sqh[:], t_h[:])
                nc.vector.reduce_sum(st2[:], sqh[:], axis=mybir.AxisListType.X)
                nc.vector.tensor_scalar_mul(mu[:], st1[:], 1.0 / H)
                nc.vector.tensor_scalar_mul(ex2[:], st2[:], 1.0 / H)
                nc.vector.tensor_mul(var[:], mu[:], mu[:])
                nc.vector.tensor_sub(var[:], ex2[:], var[:])
                nc.scalar.activation(sd[:], var[:], AF.Sqrt, bias=t_lneps[:])
                nc.vector.reciprocal_approx_fast(out=rstd[:], in_=sd[:])
                for g in range(G):
                    nc.vector.tensor_scalar(nrm[:, g, :], t_h[:, g, :],
                                            mu[:, g:g + 1], rstd[:, g:g + 1],
                                            ALU.subtract, ALU.mult)
                if not ln_triv[l]:
                    nc.vector.tensor_mul(
                        nrm[:], nrm[:],
                        t_lnrep[:, l * 2 * H:l * 2 * H + H]
                        .rearrange("p h -> p 1 h").to_broadcast([128, G, H]))
                    nc.vector.tensor_add(
                        nrm[:], nrm[:],
                        t_lnrep[:, l * 2 * H + H:(l + 1) * 2 * H]
                        .rearrange("p h -> p 1 h").to_broadcast([128, G, H]))
                nc.scalar.activation(t_z[:], nrm[:], AF.Relu)
                nc.vector.tensor_scalar_add(t_zp[:], t_z[:], MSG_EPS)
                nc.scalar.activation(t_ez[:], t_zp[:], AF.Exp, scale=t_l)
                nc.vector.tensor_copy(t_ezq[:, :, 0:H], t_ez[:])
                nc.vector.tensor_mul(t_ezq[:, :, H:H2], t_ez[:], t_zp[:])

                # ===== phase B: share [ez|q] =============================
                for g in range(G):
                    r = rows_of[g]
                    nc.sync.dma_start(cc_in[128 * g:128 * g + r, :],
                                      t_ezq[0:r, g, :])
                nc.gpsimd.collective_compute(
                    "AllGather", ALU.bypass,
                    replica_groups=[list(range(NCORES))],
                    ins=[cc_in[:]], outs=[cc_out[:]])

                # ===== phase C: gather + segment matmuls =================
                for g in range(G):
                    npg = cpg[g]
                    o = off[g]
                    nc.gpsimd.dma_gather(
                        out_ap=t_edges[:, o:o + npg, :],
                        in_ap=cc_out[:],
                        idxs_ap=t_idx[:, o * 8:(o + npg) * 8],
                        num_idxs=npg * 128,
                        num_idxs_reg=npg * 128,
                        elem_size=H2,
                        single_packet=False,
                        queue_num=g % 4)
                    ps_g = pseg.tile([128, H2], F32, tag="pseg")
                    for j in range(npg):
                        nc.tensor.matmul(ps_g[:], lhsT=t_s[:, o + j, :],
                                         rhs=t_edges[:, o + j, :],
                                         start=(j == 0), stop=(j == npg - 1))
                    nc.scalar.copy(t_dn[:, g, :], ps_g[:])

                # ===== phase D: agg + MLP ================================
                nc.vector.tensor_scalar_add(t_dne[:], t_dn[:, :, 0:H], SM_EPS)
                nc.vector.reciprocal_approx_fast(out=t_rec[:], in_=t_dne[:])
                nc.vector.tensor_mul(t_tmp[:], t_dn[:, :, H:H2], t_rec[:])
                nc.vector.tensor_add(t_outsb[:], t_tmp[:], t_z[:])

                t_ps1 = ps1p.tile([128, G, H2], F32, tag="ps1")
                for g in range(G):
                    psT = pst.tile([H, 128], F32, tag="pst")
                    nc.tensor.transpose(psT[:], t_outsb[:, g, :], t_id[:])
                    nc.scalar.copy(t_outT[:, g, :], psT[:])
                    nc.tensor.matmul(t_ps1[:, g, :], lhsT=t_outT[:, g, :],
                                     rhs=w1_l, start=True, stop=b1_triv[l])
                    if not b1_triv[l]:
                        rank1_bias(t_ps1[:, g, :],
                                   t_brows[:, l * H2:(l + 1) * H2], H2)

                st1b = stat.tile([128, G], F32, tag="st1b")
                st2b = stat.tile([128, G], F32, tag="st2b")
                mu2 = stat.tile([128, G], F32, tag="mu2")
                ex2b = stat.tile([128, G], F32, tag="ex2b")
                var2 = stat.tile([128, G], F32, tag="var2")
                sd2 = stat.tile([128, G], F32, tag="sd2")
                rstd2 = stat.tile([128, G], F32, tag="rstd2")

                nc.vector.reduce_sum(st1b[:], t_ps1[:], axis=mybir.AxisListType.X)
                nc.scalar.square(t_sq2[:], t_ps1[:])
                nc.vector.reduce_sum(st2b[:], t_sq2[:], axis=mybir.AxisListType.X)
                nc.vector.tensor_scalar_mul(mu2[:], st1b[:], 1.0 / H2)
                nc.vector.tensor_scalar_mul(ex2b[:], st2b[:], 1.0 / H2)
                nc.vector.tensor_mul(var2[:], mu2[:], mu2[:])
                nc.vector.tensor_sub(var2[:], ex2b[:], var2[:])
                nc.scalar.activation(sd2[:], var2[:], AF.Sqrt, bias=t_lneps[:])
                nc.vector.reciprocal_approx_fast(out=rstd2[:], in_=sd2[:])
                for g in range(G):
                    nc.vector.tensor_scalar(t_nrm2[:, g, :], t_ps1[:, g, :],
                                            mu2[:, g:g + 1], rstd2[:, g:g + 1],
                                            ALU.subtract, ALU.mult)
                if not mlp_triv[l]:
                    nc.vector.tensor_mul(
                        t_nrm2[:], t_nrm2[:],
                        t_mlprep[:, l * 2 * H2:l * 2 * H2 + H2]
                        .rearrange("p h -> p 1 h").to_broadcast([128, G, H2]))
                    nc.vector.tensor_add(
                        t_nrm2[:], t_nrm2[:],
                        t_mlprep[:, l * 2 * H2 + H2:(l + 1) * 2 * H2]
                        .rearrange("p h -> p 1 h").to_broadcast([128, G, H2]))
                nc.scalar.activation(t_u[:], t_nrm2[:], AF.Relu)

                for g in range(G):
                    psT2 = pst.tile([128, 128], F32, tag="pst")
                    nc.tensor.transpose(psT2[:], t_u[:, g, :], t_id[:])
                    if g % 2 == 0:
                        nc.vector.tensor_copy(t_uT[:, g, :], psT2[:])
                    else:
                        nc.scalar.copy(t_uT[:, g, :], psT2[:])
                    ps2 = pst.tile([128, H], F32, tag="pst")
                    nc.tensor.matmul(ps2[:], lhsT=t_uT[:, g, :], rhs=w2_l,
                                     start=True, stop=b2_triv[l])
                    if not b2_triv[l]:
                        rank1_bias(ps2[:], t_brows[:, L * H2 + l * H:L * H2 + (l + 1) * H], H)
                    nc.vector.tensor_add(t_h[:, g, :], t_h[:, g, :], ps2[:])

            # ---- head: out = h @ head_W (+head_b) -----------------------
            for g in range(G):
                psT = pst.tile([H, 128], F32, tag="pst")
                nc.tensor.transpose(psT[:], t_h[:, g, :], t_id[:])
                nc.scalar.copy(t_outT[:, g, :], psT[:])
                ps_o = pst.tile([128, OUT], F32, tag="pst")
                nc.tensor.matmul(ps_o[:], lhsT=t_outT[:, g, :], rhs=t_headw[:],
                                 start=True, stop=headb_triv)
                if not headb_triv:
                    rank1_bias(ps_o[:], t_brows[:, L * H2 + L * H + H:], OUT)
                ou = stat.tile([128, OUT], F32, tag="ou")
                nc.vector.tensor_copy(ou[:], ps_o[:])
                r = rows_of[g]
                nc.sync.dma_start(p_out[128 * g:128 * g + r, :], ou[0:r, :])

    nc.compile()
    return nc


_CACHE = {}


def kernel(**inputs):
    x = np.asarray(inputs["x"], np.float32)
    edge_index = np.asarray(inputs["edge_index"]).astype(np.int64)
    params = {k: np.asarray(v, np.float32) for k, v in inputs.items()
              if k not in ("x", "edge_index")}

    cpg, off, tc, idx_tiles, s_tiles = _preprocess(edge_index)

    key = (tc, tuple(cpg),
           tuple(np.asarray(inputs["t"], np.float32).tolist()))
    nc = _CACHE.get(key)
    if nc is None:
        nc = _build(params, cpg, off, tc)
        _CACHE[key] = nc

    w1 = params["W1"][:L].transpose(1, 0, 2).reshape(H, L * H2)
    w2 = params["W2"][:L].transpose(1, 0, 2).reshape(H2, L * H)
    id128 = np.eye(128, dtype=np.float32)

    brows = np.concatenate([params["b1"][:L].reshape(-1), params["b2"][:L].reshape(-1),
                            params["enc_b"].reshape(-1),
                            params["head_b"].reshape(-1)])[None, :].astype(np.float32)
    lnrep = np.tile(np.concatenate([params["ln_g"][:L], params["ln_b"][:L]], axis=1)
                    .reshape(1, L * 2 * H), (128, 1)).astype(np.float32)
    mlprep = np.tile(np.concatenate([params["mlp_g"][:L], params["mlp_b"][:L]], axis=1)
                     .reshape(1, L * 2 * H2), (128, 1)).astype(np.float32)

    in_maps = []
    for k in range(NCORES):
        xs = x[k * NPC:(k + 1) * NPC]
        xT = np.zeros((128, G * 128), np.float32)
        xT[:, :NPC] = xs.T
        m = {
            "xT": xT,
            "s": s_tiles[k],
            "idx": idx_tiles[k],
            "w1": np.ascontiguousarray(w1),
            "w2": np.ascontiguousarray(w2),
            "encw": params["enc_W"],
            "headw": params["head_W"],
            "id128": id128,
        }
        names = {t.name for t in nc.m.functions[0].allocations
                 if hasattr(t, "name")}
        # optional params only exist when the build emitted them
        for nm, arr in (("brows", brows), ("lnrep", lnrep), ("mlprep", mlprep)):
            m[nm] = arr
        in_maps.append(m)

    # drop optional inputs the program doesn't declare
    declared = set()
    for alloc in nc.m.functions[0].allocations:
        if isinstance(alloc, mybir.MemoryLocationSet) and alloc.kind == "ExternalInput":
            declared.add(alloc.memorylocations[0].name)
    in_maps = [{k2: v for k2, v in m.items() if k2 in declared} for m in in_maps]

    if os.environ.get("GNN_SIM", "0") == "1":
        from concourse.bass_interp import MultiCoreSim
        sim = MultiCoreSim(nc, NCORES)
        for k in range(NCORES):
            for name, arr in in_maps[k].items():
                sim.cores[k].tensor(name)[:] = arr
        sim.simulate()
        out = np.concatenate([np.asarray(sim.cores[k].tensor("out"))
                              for k in range(NCORES)], axis=0)
        return out.astype(np.float32)

    trace = os.environ.get("GNN_TRACE", "0") == "1"
    res = run_bass_kernel_spmd(nc, in_maps, list(range(NCORES)), trace=trace)
    if trace and res.exec_time_ns is not None:
        print(f"HW exec time: {res.exec_time_ns} ns")

    out = np.concatenate([res.results[k]["out"] for k in range(NCORES)], axis=0)
    return out.astype(np.float32)
